# revision 5
# baseline (speedup 1.0000x reference)
"""Trainium2 Bass kernel: 3x3 same-padding conv, x[1,16,1024,1024] f32.

Strategy: shard H across 8 cores (128 output rows each; host supplies the
1-row halo by overlapping shards of a zero-padded input, so no collectives).

Per-core kernel (implicit GEMM with H-block output packing):
  - SBUF layout: partition p = u*16 + c  (u in 0..7 = input-row window slot,
    c = input channel).  Partition (u,c), slot k holds local input row
    6k+u of channel c in columns 1..1024 of a 1026-wide slot (cols 0/1025 are
    zero guards for the kx = x-shift taps).  The HOST pre-gathers the shard
    into exactly this [128, 22*1026] layout (guards included), so the input
    DMA is a flat per-partition-contiguous copy at full DMA efficiency.
  - Output rows are processed in blocks of j=6 rows: out row y = rk + j,
    rk = 6k (k=0..20) and rk=122 for the tail block k=21.
    Tap (ky,kx) of output row j needs local input row rk + (j+ky) = rk + u,
    so all 3x3 taps of a block read slot k only; u = j+ky spans 0..7.
  - matmul: out[(j,co), x] = sum_{(u,c)} lhsT_kx[(u,c),(j,co)] * X[(u,c), x+kx]
    with lhsT_kx[(u,c),(j,co)] = W[co,c,u-j,kx] if 0<=u-j<=2 else 0.
    K=128 (full contraction), M=96, N=512 (two halves per row-block).
    3 accumulating matmuls per PSUM tile (one per kx, free-dim shifted rhs).
    float32r (reduced-precision fp32) streams at 1 cycle/row vs 4 for fp32.
  - PSUM -> SBUF eviction adds bias (per-partition scalar), alternating
    between DVE (tensor_scalar_add) and ACT (activation Identity+bias).
  - Output is staged 3 blocks per SBUF tile and DMA'd flat to a permuted
    y buffer [96, 22528] (host un-gathers); out-DMAs ride the second HWDGE
    ring (nc.scalar) so input and output streams don't share a FIFO.
"""

import sys

sys.path.insert(0, "/opt/trn_rl_repo")

import numpy as np

N_CORES = 8
C = 16            # channels in/out
H = 1024
W = 1024
HSH = H // N_CORES  # 128 output rows per core
HL = HSH + 2      # local input rows incl. halo
J = 6             # output rows per block
U = 8             # input-row window per block (J + 2)
SLOT = W + 2      # 1026, row slot width with zero guards
NBLK = 21         # full blocks at rk = 6k
TAIL_RK = HSH - J  # 122, tail block start
NSLOT = NBLK + 1  # 22 slots per partition
M = J * C         # 96 output partitions (j, cout)
NHALF = 512
CHUNKS = [(0, 2), (2, 4), (6, 4), (10, 4), (14, 4), (18, 4)]  # (slot start, n)
XCOLS = NSLOT * SLOT          # 22572 per-partition input columns
NGRP = 7                      # out groups of 3 blocks (+ tail)
YCOLS = NGRP * 3 * W + W      # 22528 per-partition output columns

_CACHE = {}


def _build(reps=1, loop_n=None, parts=('in', 'mm', 'ev', 'out')):
    import contextlib

    import concourse.bacc as bacc
    import concourse.tile as tile
    import concourse.mybir as mybir

    f32 = mybir.dt.float32
    bf16 = mybir.dt.bfloat16

    nc = bacc.Bacc("TRN2", target_bir_lowering=False, debug=False,
                   num_devices=N_CORES)

    x_d = nc.dram_tensor("x", [128, XCOLS], bf16, kind="ExternalInput")
    w_d = [nc.dram_tensor(f"w{kx}", [128, M], bf16, kind="ExternalInput")
           for kx in range(3)]
    b_d = nc.dram_tensor("bvec", [M, 1], f32, kind="ExternalInput")
    y_d = nc.dram_tensor("y", [M, YCOLS], bf16, kind="ExternalOutput")

    with tile.TileContext(nc) as tc:
        with (
            tc.tile_pool(name="xpool", bufs=1) as xpool,
            tc.tile_pool(name="wpool", bufs=1) as wpool,
            tc.tile_pool(name="opool", bufs=4) as opool,
            tc.tile_pool(name="pspool", bufs=8, space="PSUM") as pspool,
        ):
            wt = []
            for kx in range(3):
                t = wpool.tile([128, M], bf16, tag=f"w{kx}")
                nc.sync.dma_start(t[:], w_d[kx].ap())
                wt.append(t[:])
            bt = wpool.tile([M, 1], f32, tag="bias")
            nc.sync.dma_start(bt[:], b_d.ap())

            ident = mybir.ActivationFunctionType.Identity
            xt = []
            for ci, (k0, ns) in enumerate(CHUNKS):
                xc = xpool.tile([128, ns * SLOT], bf16, tag=f"x{ci}")
                xt.append(xc)

            def chunk_of(k):
                for ci, (k0, ns) in enumerate(CHUNKS):
                    if k0 <= k < k0 + ns:
                        return ci, k0
                raise AssertionError(k)

            loop_cm = (tc.For_i(0, loop_n, 1) if loop_n is not None
                       else contextlib.nullcontext())
            with loop_cm:
              for _ in range(reps):
                if 'in' in parts:
                    # flat per-partition-contiguous chunk loads
                    for ci, (k0, ns) in enumerate(CHUNKS):
                        nc.sync.dma_start(
                            xt[ci][:],
                            x_d.ap()[:, k0 * SLOT:(k0 + ns) * SLOT])

                for g in range(NGRP + 1):  # groups of 3 blocks; last = tail
                    blocks = ([3 * g, 3 * g + 1, 3 * g + 2] if g < NGRP
                              else [NBLK])
                    gw = len(blocks) * W
                    og = opool.tile([M, gw], bf16, tag=f"o{g % 2}_{gw}")
                    for bi, k in enumerate(blocks):
                        ci, k0 = chunk_of(k)
                        X2 = xt[ci][:]
                        for h in range(2):
                            base = (k - k0) * SLOT + h * NHALF
                            ps = pspool.tile([M, NHALF], f32)
                            if 'mm' in parts:
                                for kx in range(3):
                                    rhs = X2[:, base + kx:base + kx + NHALF]
                                    nc.tensor.matmul(ps[:], wt[kx], rhs,
                                                     start=(kx == 0),
                                                     stop=(kx == 2))
                            dst_ev = og[:, bi * W + h * NHALF:
                                        bi * W + h * NHALF + NHALF]
                            if 'ev' in parts:
                                if (2 * k + h) % 2 == 0:
                                    nc.vector.tensor_scalar_add(
                                        dst_ev, ps[:], bt[:])
                                else:
                                    nc.scalar.activation(dst_ev, ps[:],
                                                         ident, bias=bt[:])
                    if 'out' in parts:
                        # flat store into the permuted y buffer.  Out-DMAs
                        # ride the (otherwise idle) GpSimd DGE ring: an
                        # out-DMA instruction blocks its sequencer until the
                        # staging tile's evictions land, and on the sync ring
                        # that stalled the next iteration's input loads
                        # (~10us/iter of PE idle at the loop boundary).
                        eng = nc.gpsimd
                        if g < NGRP:
                            eng.dma_start(
                                y_d.ap()[:, g * 3 * W:(g + 1) * 3 * W],
                                og[:])
                        else:
                            # tail block: only rows 126,127 (j=4,5) are new
                            eng.dma_start(
                                y_d.ap()[4 * C:6 * C, NGRP * 3 * W:],
                                og[4 * C:6 * C, :])

    nc.compile()
    return nc


def _bf16():
    import ml_dtypes

    return ml_dtypes.bfloat16


def _prep_weights(weight, bias):
    # lhsT_kx[(u,c),(j,co)] = W[co,c,u-j,kx] for 0<=u-j<=2
    wts = []
    for kx in range(3):
        wk = np.zeros((128, M), dtype=np.float32)
        for ky in range(3):
            wcc = np.ascontiguousarray(weight[:, :, ky, kx].T)  # [c, co]
            for j in range(J):
                u = j + ky
                wk[u * C:(u + 1) * C, j * C:(j + 1) * C] = wcc
        wts.append(wk.astype(_bf16()))
    bvec = np.tile(bias.astype(np.float32), J)[:, None].copy()
    return wts, bvec


def _make_in_maps(x, weight, bias):
    # zero-padded input in [row, channel, W] order, quantized to bf16 once
    x_pad = np.zeros((H + 2, C, W), dtype=_bf16())
    x_pad[1:H + 1] = x[0].transpose(1, 0, 2).astype(_bf16())
    wts, bvec = _prep_weights(weight, bias)

    in_maps = []
    for s in range(N_CORES):
        # pre-gathered shard: partition p = u*16+c, slot k, cols 1..1024
        # hold local input row 6k+u (k<21) / 122+u (k=21) of channel c.
        xs = np.zeros((U, C, NSLOT, SLOT), dtype=_bf16())
        r0 = s * HSH
        for u in range(U):
            # rows r0+6k+u for k=0..20 -> strided slice, [21, C, W]
            xs[u, :, :NBLK, 1:W + 1] = x_pad[
                r0 + u:r0 + u + 6 * NBLK:6].transpose(1, 0, 2)
            xs[u, :, NBLK, 1:W + 1] = x_pad[r0 + TAIL_RK + u]
        m = {"x": xs.reshape(128, XCOLS), "bvec": bvec}
        for kx in range(3):
            m[f"w{kx}"] = wts[kx]
        in_maps.append(m)
    return in_maps


def _gather_out(results):
    out = np.empty((C, H, W), dtype=np.float32)
    for s in range(N_CORES):
        yp = results[s]["y"].astype(np.float32)  # [96, 22528]
        # main: rows 0..125 = (g, b, j) lexicographic
        main = yp[:, :NGRP * 3 * W].reshape(J, C, NGRP * 3, W)
        out[:, s * HSH:s * HSH + 126] = (
            main.transpose(1, 2, 0, 3).reshape(C, 126, W))
        # tail: rows 126, 127 from j = 4, 5
        tail = yp[:, NGRP * 3 * W:].reshape(J, C, W)[4:6]
        out[:, s * HSH + 126:s * HSH + 128] = tail.transpose(1, 0, 2)
    return out


def get_nc(reps=1, loop_n=None, parts=('in', 'mm', 'ev', 'out')):
    key = f"nc{reps}_{loop_n}_{parts}"
    if key not in _CACHE:
        _CACHE[key] = _build(reps, loop_n, parts)
    return _CACHE[key]


def kernel(x, weight, bias):
    x = np.asarray(x, dtype=np.float32)
    weight = np.asarray(weight, dtype=np.float32)
    bias = np.asarray(bias, dtype=np.float32)

    nc = get_nc()

    from concourse.bass_utils import run_bass_kernel_spmd

    in_maps = _make_in_maps(x, weight, bias)
    res = run_bass_kernel_spmd(nc, in_maps, list(range(N_CORES)))
    return _gather_out(res.results)



# revision 7
# speedup vs baseline: 1.0159x; 1.0159x over previous
"""Trainium2 Bass kernel: 3x3 same-padding conv, x[1,16,1024,1024] f32.

Strategy: shard H across 8 cores (128 output rows each; host supplies the
1-row halo by overlapping shards of a zero-padded input, so no collectives).

Per-core kernel (implicit GEMM with H-block output packing):
  - SBUF layout: partition p = u*16 + c  (u in 0..7 = input-row window slot,
    c = input channel).  Partition (u,c), slot k holds local input row
    6k+u of channel c in columns 1..1024 of a 1026-wide slot (cols 0/1025 are
    zero guards for the kx = x-shift taps).  The HOST pre-gathers the shard
    into exactly this [128, 22*1026] layout (guards included), so the input
    DMA is a flat per-partition-contiguous copy at full DMA efficiency.
  - Output rows are processed in blocks of j=6 rows: out row y = rk + j,
    rk = 6k (k=0..20) and rk=122 for the tail block k=21.
    Tap (ky,kx) of output row j needs local input row rk + (j+ky) = rk + u,
    so all 3x3 taps of a block read slot k only; u = j+ky spans 0..7.
  - matmul: out[(j,co), x] = sum_{(u,c)} lhsT_kx[(u,c),(j,co)] * X[(u,c), x+kx]
    with lhsT_kx[(u,c),(j,co)] = W[co,c,u-j,kx] if 0<=u-j<=2 else 0.
    K=128 (full contraction), M=96, N=512 (two halves per row-block).
    3 accumulating matmuls per PSUM tile (one per kx, free-dim shifted rhs).
    float32r (reduced-precision fp32) streams at 1 cycle/row vs 4 for fp32.
  - PSUM -> SBUF eviction adds bias (per-partition scalar), alternating
    between DVE (tensor_scalar_add) and ACT (activation Identity+bias).
  - Output is staged 3 blocks per SBUF tile and DMA'd flat to a permuted
    y buffer [96, 22528] (host un-gathers); out-DMAs ride the second HWDGE
    ring (nc.scalar) so input and output streams don't share a FIFO.
"""

import sys

sys.path.insert(0, "/opt/trn_rl_repo")

import numpy as np

N_CORES = 8
C = 16            # channels in/out
H = 1024
W = 1024
HSH = H // N_CORES  # 128 output rows per core
HL = HSH + 2      # local input rows incl. halo
J = 6             # output rows per block
U = 8             # input-row window per block (J + 2)
SLOT = W + 2      # 1026, row slot width with zero guards
NBLK = 21         # full blocks at rk = 6k
TAIL_RK = HSH - J  # 122, tail block start
NSLOT = NBLK + 1  # 22 slots per partition
M = J * C         # 96 output partitions (j, cout)
NHALF = 512
# (slot start, n): fine-grained leading chunks so the first matmuls start
# ~3us after the For_i boundary instead of waiting for a 2-4 slot transfer
CHUNKS = [(0, 1), (1, 1), (2, 2), (4, 2), (6, 4), (10, 4), (14, 4), (18, 4)]
XCOLS = NSLOT * SLOT          # 22572 per-partition input columns
NGRP = 7                      # out groups of 3 blocks (+ tail)
YCOLS = NGRP * 3 * W + W      # 22528 per-partition output columns

_CACHE = {}


def _build(reps=1, loop_n=None, parts=('in', 'mm', 'ev', 'out')):
    import contextlib

    import concourse.bacc as bacc
    import concourse.tile as tile
    import concourse.mybir as mybir

    f32 = mybir.dt.float32
    bf16 = mybir.dt.bfloat16

    nc = bacc.Bacc("TRN2", target_bir_lowering=False, debug=False,
                   num_devices=N_CORES)

    x_d = nc.dram_tensor("x", [128, XCOLS], bf16, kind="ExternalInput")
    w_d = [nc.dram_tensor(f"w{kx}", [128, M], bf16, kind="ExternalInput")
           for kx in range(3)]
    b_d = nc.dram_tensor("bvec", [M, 1], f32, kind="ExternalInput")
    y_d = nc.dram_tensor("y", [M, YCOLS], bf16, kind="ExternalOutput")

    with tile.TileContext(nc) as tc:
        with (
            tc.tile_pool(name="xpool", bufs=1) as xpool,
            tc.tile_pool(name="wpool", bufs=1) as wpool,
            tc.tile_pool(name="opool", bufs=4) as opool,
            tc.tile_pool(name="pspool", bufs=8, space="PSUM") as pspool,
        ):
            wt = []
            for kx in range(3):
                t = wpool.tile([128, M], bf16, tag=f"w{kx}")
                nc.sync.dma_start(t[:], w_d[kx].ap())
                wt.append(t[:])
            bt = wpool.tile([M, 1], f32, tag="bias")
            nc.sync.dma_start(bt[:], b_d.ap())

            ident = mybir.ActivationFunctionType.Identity
            xt = []
            for ci, (k0, ns) in enumerate(CHUNKS):
                xc = xpool.tile([128, ns * SLOT], bf16, tag=f"x{ci}")
                xt.append(xc)

            def chunk_of(k):
                for ci, (k0, ns) in enumerate(CHUNKS):
                    if k0 <= k < k0 + ns:
                        return ci, k0
                raise AssertionError(k)

            loop_cm = (tc.For_i(0, loop_n, 1) if loop_n is not None
                       else contextlib.nullcontext())
            with loop_cm:
              for _ in range(reps):
                if 'in' in parts:
                    # flat per-partition-contiguous chunk loads
                    for ci, (k0, ns) in enumerate(CHUNKS):
                        nc.sync.dma_start(
                            xt[ci][:],
                            x_d.ap()[:, k0 * SLOT:(k0 + ns) * SLOT])

                for g in range(NGRP + 1):  # groups of 3 blocks; last = tail
                    blocks = ([3 * g, 3 * g + 1, 3 * g + 2] if g < NGRP
                              else [NBLK])
                    gw = len(blocks) * W
                    og = opool.tile([M, gw], bf16, tag=f"o{g % 2}_{gw}")
                    for bi, k in enumerate(blocks):
                        ci, k0 = chunk_of(k)
                        X2 = xt[ci][:]
                        for h in range(2):
                            base = (k - k0) * SLOT + h * NHALF
                            ps = pspool.tile([M, NHALF], f32)
                            if 'mm' in parts:
                                for kx in range(3):
                                    rhs = X2[:, base + kx:base + kx + NHALF]
                                    nc.tensor.matmul(ps[:], wt[kx], rhs,
                                                     start=(kx == 0),
                                                     stop=(kx == 2))
                            if 'ev' in parts:
                                # tail block: only j=4,5 (rows 126,127) are
                                # new -> evict just those 32 partitions to
                                # shorten the end-of-iteration chain
                                p0, p1 = (4 * C, 6 * C) if k == NBLK else (0, M)
                                dst_ev = og[p0:p1, bi * W + h * NHALF:
                                            bi * W + h * NHALF + NHALF]
                                if (2 * k + h) % 2 == 0:
                                    nc.vector.tensor_scalar_add(
                                        dst_ev, ps[p0:p1], bt[p0:p1])
                                else:
                                    nc.scalar.activation(dst_ev, ps[p0:p1],
                                                         ident, bias=bt[p0:p1])
                    if 'out' in parts:
                        # flat store into the permuted y buffer.  Out-DMAs
                        # ride the (otherwise idle) GpSimd DGE ring: an
                        # out-DMA instruction blocks its sequencer until the
                        # staging tile's evictions land, and on the sync ring
                        # that stalled the next iteration's input loads
                        # (~10us/iter of PE idle at the loop boundary).
                        eng = nc.gpsimd
                        if g < NGRP:
                            eng.dma_start(
                                y_d.ap()[:, g * 3 * W:(g + 1) * 3 * W],
                                og[:])
                        else:
                            # tail block: only rows 126,127 (j=4,5) are new
                            eng.dma_start(
                                y_d.ap()[4 * C:6 * C, NGRP * 3 * W:],
                                og[4 * C:6 * C, :])

    nc.compile()
    return nc


def _bf16():
    import ml_dtypes

    return ml_dtypes.bfloat16


def _prep_weights(weight, bias):
    # lhsT_kx[(u,c),(j,co)] = W[co,c,u-j,kx] for 0<=u-j<=2
    wts = []
    for kx in range(3):
        wk = np.zeros((128, M), dtype=np.float32)
        for ky in range(3):
            wcc = np.ascontiguousarray(weight[:, :, ky, kx].T)  # [c, co]
            for j in range(J):
                u = j + ky
                wk[u * C:(u + 1) * C, j * C:(j + 1) * C] = wcc
        wts.append(wk.astype(_bf16()))
    bvec = np.tile(bias.astype(np.float32), J)[:, None].copy()
    return wts, bvec


def _make_in_maps(x, weight, bias):
    # zero-padded input in [row, channel, W] order, quantized to bf16 once
    x_pad = np.zeros((H + 2, C, W), dtype=_bf16())
    x_pad[1:H + 1] = x[0].transpose(1, 0, 2).astype(_bf16())
    wts, bvec = _prep_weights(weight, bias)

    in_maps = []
    for s in range(N_CORES):
        # pre-gathered shard: partition p = u*16+c, slot k, cols 1..1024
        # hold local input row 6k+u (k<21) / 122+u (k=21) of channel c.
        xs = np.zeros((U, C, NSLOT, SLOT), dtype=_bf16())
        r0 = s * HSH
        for u in range(U):
            # rows r0+6k+u for k=0..20 -> strided slice, [21, C, W]
            xs[u, :, :NBLK, 1:W + 1] = x_pad[
                r0 + u:r0 + u + 6 * NBLK:6].transpose(1, 0, 2)
            xs[u, :, NBLK, 1:W + 1] = x_pad[r0 + TAIL_RK + u]
        m = {"x": xs.reshape(128, XCOLS), "bvec": bvec}
        for kx in range(3):
            m[f"w{kx}"] = wts[kx]
        in_maps.append(m)
    return in_maps


def _gather_out(results):
    out = np.empty((C, H, W), dtype=np.float32)
    for s in range(N_CORES):
        yp = results[s]["y"].astype(np.float32)  # [96, 22528]
        # main: rows 0..125 = (g, b, j) lexicographic
        main = yp[:, :NGRP * 3 * W].reshape(J, C, NGRP * 3, W)
        out[:, s * HSH:s * HSH + 126] = (
            main.transpose(1, 2, 0, 3).reshape(C, 126, W))
        # tail: rows 126, 127 from j = 4, 5
        tail = yp[:, NGRP * 3 * W:].reshape(J, C, W)[4:6]
        out[:, s * HSH + 126:s * HSH + 128] = tail.transpose(1, 0, 2)
    return out


def get_nc(reps=1, loop_n=None, parts=('in', 'mm', 'ev', 'out')):
    key = f"nc{reps}_{loop_n}_{parts}"
    if key not in _CACHE:
        _CACHE[key] = _build(reps, loop_n, parts)
    return _CACHE[key]


def kernel(x, weight, bias):
    x = np.asarray(x, dtype=np.float32)
    weight = np.asarray(weight, dtype=np.float32)
    bias = np.asarray(bias, dtype=np.float32)

    nc = get_nc()

    from concourse.bass_utils import run_bass_kernel_spmd

    in_maps = _make_in_maps(x, weight, bias)
    res = run_bass_kernel_spmd(nc, in_maps, list(range(N_CORES)))
    return _gather_out(res.results)



# revision 11
# speedup vs baseline: 1.0929x; 1.0758x over previous
"""Trainium2 Bass kernel: 3x3 same-padding conv, x[1,16,1024,1024] f32.

Strategy: shard H across 8 cores (128 output rows each; host supplies the
1-row halo by overlapping shards of a zero-padded input, so no collectives).

Per-core kernel (implicit GEMM with H-block output packing):
  - SBUF layout: partition p = u*16 + c  (u in 0..7 = input-row window slot,
    c = input channel).  Partition (u,c), slot k holds local input row
    6k+u of channel c in columns 1..1024 of a 1026-wide slot (cols 0/1025 are
    zero guards for the kx = x-shift taps).  The HOST pre-gathers the shard
    into exactly this [128, 22*1026] layout (guards included), so the input
    DMA is a flat per-partition-contiguous copy at full DMA efficiency.
  - Output rows are processed in blocks of j=6 rows: out row y = rk + j,
    rk = 6k (k=0..20) and rk=122 for the tail block k=21.
    Tap (ky,kx) of output row j needs local input row rk + (j+ky) = rk + u,
    so all 3x3 taps of a block read slot k only; u = j+ky spans 0..7.
  - matmul: out[(j,co), x] = sum_{(u,c)} lhsT_kx[(u,c),(j,co)] * X[(u,c), x+kx]
    with lhsT_kx[(u,c),(j,co)] = W[co,c,u-j,kx] if 0<=u-j<=2 else 0.
    K=128 (full contraction), M=96, N=512 (two halves per row-block).
    3 accumulating matmuls per PSUM tile (one per kx, free-dim shifted rhs).
    float32r (reduced-precision fp32) streams at 1 cycle/row vs 4 for fp32.
  - PSUM -> SBUF eviction adds bias (per-partition scalar), alternating
    between DVE (tensor_scalar_add) and ACT (activation Identity+bias).
  - Output is staged 3 blocks per SBUF tile and DMA'd flat to a permuted
    y buffer [96, 22528] (host un-gathers); out-DMAs ride the second HWDGE
    ring (nc.scalar) so input and output streams don't share a FIFO.
"""

import sys

sys.path.insert(0, "/opt/trn_rl_repo")

import numpy as np

N_CORES = 8
C = 16            # channels in/out
H = 1024
W = 1024
HSH = H // N_CORES  # 128 output rows per core
HL = HSH + 2      # local input rows incl. halo
J = 6             # output rows per block
U = 8             # input-row window per block (J + 2)
SLOT = W + 2      # 1026, row slot width with zero guards
NBLK = 21         # full blocks at rk = 6k
TAIL_RK = HSH - J  # 122, tail block start
NSLOT = NBLK + 1  # 22 slots per partition
M = J * C         # 96 output partitions (j, cout)
NHALF = 512
# (slot start, n): fine-grained leading chunks so the first matmuls start
# ~3us after the For_i boundary instead of waiting for a 2-4 slot transfer.
# Slot 0 is loaded as two half-slot tiles (see HALF0) so block 0 can start
# even earlier.
CHUNKS = [(1, 1), (2, 2), (4, 2), (6, 4), (10, 4), (14, 4), (18, 4)]
HALF0 = NHALF + 2             # 514-wide half-slot tiles for slot 0
XCOLS = NSLOT * SLOT          # 22572 per-partition input columns
# output groups of blocks; trailing groups shrink so the final out-DMA
# chain after the last matmul is short (block 20 goes out per 512-half)
OGROUPS = [[0, 1, 2], [3, 4, 5], [6, 7, 8], [9, 10, 11], [12, 13, 14],
           [15, 16, 17], [18, 19], [20], [NBLK]]
YCOLS = NBLK * W + W          # 22528 per-partition output columns
NJUNK_HEAD = 6                # warm-keeper matmuls at body start
NJUNK_TAIL = 7                # warm-keeper matmuls covering the out tail

_CACHE = {}


def _build(reps=1, loop_n=None, parts=('in', 'mm', 'ev', 'out')):
    import contextlib

    import concourse.bacc as bacc
    import concourse.tile as tile
    import concourse.mybir as mybir

    f32 = mybir.dt.float32
    bf16 = mybir.dt.bfloat16

    nc = bacc.Bacc("TRN2", target_bir_lowering=False, debug=False,
                   num_devices=N_CORES)

    x_d = nc.dram_tensor("x", [128, XCOLS], bf16, kind="ExternalInput")
    w_d = [nc.dram_tensor(f"w{kx}", [128, M], bf16, kind="ExternalInput")
           for kx in range(3)]
    b_d = nc.dram_tensor("bvec", [M, 1], f32, kind="ExternalInput")
    y_d = nc.dram_tensor("y", [M, YCOLS], bf16, kind="ExternalOutput")

    with tile.TileContext(nc) as tc:
        with (
            tc.tile_pool(name="xpool", bufs=1) as xpool,
            tc.tile_pool(name="wpool", bufs=1) as wpool,
            tc.tile_pool(name="opool", bufs=4) as opool,
            tc.tile_pool(name="pspool", bufs=7, space="PSUM") as pspool,
            tc.tile_pool(name="psjpool", bufs=1, space="PSUM") as psjpool,
        ):
            wt = []
            for kx in range(3):
                t = wpool.tile([128, M], bf16, tag=f"w{kx}")
                nc.sync.dma_start(t[:], w_d[kx].ap())
                wt.append(t[:])
            bt = wpool.tile([M, 1], f32, tag="bias")
            nc.sync.dma_start(bt[:], b_d.ap())
            # scratch rhs + junk PSUM bank for warm-keeper matmuls: they run
            # while the PE would otherwise idle at the For_i boundary (out
            # tail + semaphore dance + input-DMA restart > the ~3.4us HAM
            # window, so without them every iteration restarts at ~1.2GHz)
            scratch = wpool.tile([128, NHALF], bf16, tag="scratch")
            nc.vector.memset(scratch[:], 0.0)
            psj = psjpool.tile([M, NHALF], f32, tag="psjunk")

            ident = mybir.ActivationFunctionType.Identity
            x0t = []
            for h in range(2):
                x0h = xpool.tile([128, HALF0], bf16, tag=f"x0h{h}",
                                 name=f"x0h{h}")
                x0t.append(x0h)
            xt = []
            for ci, (k0, ns) in enumerate(CHUNKS):
                xc = xpool.tile([128, ns * SLOT], bf16, tag=f"x{ci}")
                xt.append(xc)

            def chunk_of(k):
                for ci, (k0, ns) in enumerate(CHUNKS):
                    if k0 <= k < k0 + ns:
                        return ci, k0
                raise AssertionError(k)

            def junk(n):
                if 'mm' in parts:
                    for _ in range(n):
                        nc.tensor.matmul(psj[:], wt[0], scratch[:],
                                         start=True, stop=True)

            loop_cm = (tc.For_i(0, loop_n, 1) if loop_n is not None
                       else contextlib.nullcontext())
            with loop_cm:
              for _ in range(reps):
                junk(NJUNK_HEAD)
                if 'in' in parts:
                    # flat per-partition-contiguous chunk loads; slot 0 as
                    # two half-slot tiles so block 0 starts ~1us earlier
                    for h in range(2):
                        nc.sync.dma_start(
                            x0t[h][:],
                            x_d.ap()[:, h * NHALF:h * NHALF + HALF0])
                    for ci, (k0, ns) in enumerate(CHUNKS):
                        nc.sync.dma_start(
                            xt[ci][:],
                            x_d.ap()[:, k0 * SLOT:(k0 + ns) * SLOT])

                for g, blocks in enumerate(OGROUPS):
                    gw = len(blocks) * W
                    og = opool.tile([M, gw], bf16, tag=f"o{g % 2}_{gw}")
                    is_tail = blocks[0] == NBLK
                    for bi, k in enumerate(blocks):
                        for h in range(2):
                            ps = pspool.tile([M, NHALF], f32)
                            if 'mm' in parts:
                                for kx in range(3):
                                    if k == 0:
                                        rhs = x0t[h][:][:, kx:kx + NHALF]
                                    else:
                                        ci, k0 = chunk_of(k)
                                        base = (k - k0) * SLOT + h * NHALF
                                        rhs = xt[ci][:][:, base + kx:
                                                        base + kx + NHALF]
                                    nc.tensor.matmul(ps[:], wt[kx], rhs,
                                                     start=(kx == 0),
                                                     stop=(kx == 2))
                            if 'ev' in parts:
                                # tail block: only j=4,5 (rows 126,127) are
                                # new -> evict just those 32 partitions to
                                # shorten the end-of-iteration chain
                                p0, p1 = (4 * C, 6 * C) if is_tail else (0, M)
                                dst_ev = og[p0:p1, bi * W + h * NHALF:
                                            bi * W + h * NHALF + NHALF]
                                if (2 * k + h) % 2 == 0:
                                    nc.vector.tensor_scalar_add(
                                        dst_ev, ps[p0:p1], bt[p0:p1])
                                else:
                                    nc.scalar.activation(dst_ev, ps[p0:p1],
                                                         ident, bias=bt[p0:p1])
                            if ('out' in parts and len(blocks) == 1
                                    and not is_tail):
                                # single-block group: ship each 512-half as
                                # soon as it is evicted
                                nc.gpsimd.dma_start(
                                    y_d.ap()[:, blocks[0] * W + h * NHALF:
                                             blocks[0] * W + (h + 1) * NHALF],
                                    og[:, h * NHALF:(h + 1) * NHALF])
                    if is_tail:
                        junk(NJUNK_TAIL)
                    if 'out' in parts:
                        # flat store into the permuted y buffer.  Out-DMAs
                        # ride the (otherwise idle) GpSimd DGE ring so they
                        # never block the input-load ring.
                        if is_tail:
                            # tail block: only rows 126,127 (j=4,5) are new
                            nc.gpsimd.dma_start(
                                y_d.ap()[4 * C:6 * C, NBLK * W:],
                                og[4 * C:6 * C, :])
                        elif len(blocks) > 1:
                            nc.gpsimd.dma_start(
                                y_d.ap()[:, blocks[0] * W:
                                         (blocks[0] + len(blocks)) * W],
                                og[:])

    nc.compile()
    return nc


def _bf16():
    import ml_dtypes

    return ml_dtypes.bfloat16


def _prep_weights(weight, bias):
    # lhsT_kx[(u,c),(j,co)] = W[co,c,u-j,kx] for 0<=u-j<=2
    wts = []
    for kx in range(3):
        wk = np.zeros((128, M), dtype=np.float32)
        for ky in range(3):
            wcc = np.ascontiguousarray(weight[:, :, ky, kx].T)  # [c, co]
            for j in range(J):
                u = j + ky
                wk[u * C:(u + 1) * C, j * C:(j + 1) * C] = wcc
        wts.append(wk.astype(_bf16()))
    bvec = np.tile(bias.astype(np.float32), J)[:, None].copy()
    return wts, bvec


def _make_in_maps(x, weight, bias):
    # zero-padded input in [row, channel, W] order, quantized to bf16 once
    x_pad = np.zeros((H + 2, C, W), dtype=_bf16())
    x_pad[1:H + 1] = x[0].transpose(1, 0, 2).astype(_bf16())
    wts, bvec = _prep_weights(weight, bias)

    in_maps = []
    for s in range(N_CORES):
        # pre-gathered shard: partition p = u*16+c, slot k, cols 1..1024
        # hold local input row 6k+u (k<21) / 122+u (k=21) of channel c.
        xs = np.zeros((U, C, NSLOT, SLOT), dtype=_bf16())
        r0 = s * HSH
        for u in range(U):
            # rows r0+6k+u for k=0..20 -> strided slice, [21, C, W]
            xs[u, :, :NBLK, 1:W + 1] = x_pad[
                r0 + u:r0 + u + 6 * NBLK:6].transpose(1, 0, 2)
            xs[u, :, NBLK, 1:W + 1] = x_pad[r0 + TAIL_RK + u]
        m = {"x": xs.reshape(128, XCOLS), "bvec": bvec}
        for kx in range(3):
            m[f"w{kx}"] = wts[kx]
        in_maps.append(m)
    return in_maps


def _gather_out(results):
    out = np.empty((C, H, W), dtype=np.float32)
    for s in range(N_CORES):
        yp = results[s]["y"].astype(np.float32)  # [96, 22528]
        # main: rows 0..125 = (block, j) lexicographic
        main = yp[:, :NBLK * W].reshape(J, C, NBLK, W)
        out[:, s * HSH:s * HSH + 126] = (
            main.transpose(1, 2, 0, 3).reshape(C, 126, W))
        # tail: rows 126, 127 from j = 4, 5
        tail = yp[:, NBLK * W:].reshape(J, C, W)[4:6]
        out[:, s * HSH + 126:s * HSH + 128] = tail.transpose(1, 0, 2)
    return out


def get_nc(reps=1, loop_n=None, parts=('in', 'mm', 'ev', 'out')):
    key = f"nc{reps}_{loop_n}_{parts}"
    if key not in _CACHE:
        _CACHE[key] = _build(reps, loop_n, parts)
    return _CACHE[key]


def kernel(x, weight, bias):
    x = np.asarray(x, dtype=np.float32)
    weight = np.asarray(weight, dtype=np.float32)
    bias = np.asarray(bias, dtype=np.float32)

    nc = get_nc()

    from concourse.bass_utils import run_bass_kernel_spmd

    in_maps = _make_in_maps(x, weight, bias)
    res = run_bass_kernel_spmd(nc, in_maps, list(range(N_CORES)))
    return _gather_out(res.results)

